# revision 55
# baseline (speedup 1.0000x reference)
"""Trainium2 Bass kernel for 12-head causal MHA (B=4, S=2048, D=768).

Sharding: 8 cores, core c -> (batch c//2, head-half c%2).  Each core
computes 6 heads over ALL 2048 queries of its batch and emits the
PARTIAL out-projection (its 384 ctx dims x woT slice); the host sums
the two half-partials per batch and adds the bias.  This removes the
K/V-projection duplication of batch x query-parity sharding and makes
queries contiguous (simple causal masks).

Layout is fully transposed so every matmul contracts along partitions:
  qT/kT: [head_dim, seq]  scoresT: [sk, sq]  ctxT: [hd+1, sq]
The softmax row-sum is fused into the ctx matmul via a ones column
appended to V (M=65).  Softmax skips max-subtraction (scores/8 are
bounded by ~2 for this distribution, exp is safe).

Schedule: projection jobs (512-key groups), attention streams (one
head-pair x 256-query block) and the out-projection are threaded into
one instruction stream so the PE never idles long enough to drop out of
its max p-state and the scalar engine's exp backlog drains during
projection bursts.  The attention inner loop is software-pipelined with
lookahead 2 (ctx of pair p-2 issues after the scores of pair p, hiding
the exp+mask latency), scores matmuls pack both heads into PE quadrant
pairs (tile_position row split), and causal masking multiplies a single
k<=u triangle on the (otherwise idle) gpsimd engine.  Softmax
normalization (bf16 row-sum cast -> rank-1 broadcast matmul ->
full-tile approx reciprocal -> scale) is deferred by at least one full
stream/job so its only tensor instruction never stalls the PE queue.
"""

import os
import sys
from contextlib import ExitStack

import numpy as np

os.environ.setdefault("MYCRO_LOCAL_CACHE", "1")

for _p in ("/root/.axon_site/_ro/trn_rl_repo", "/opt/trn_rl_repo"):
    # later inserts win: prefer /opt (writable sibling modules, e.g.
    # antenv.axon_hooks) over the read-only mirror
    if os.path.isdir(_p) and _p not in sys.path:
        sys.path.insert(0, _p)

import concourse.bass as bass  # noqa: E402
import concourse.tile as tile  # noqa: E402
from concourse import bacc, mybir  # noqa: E402
from concourse.bass_utils import run_bass_kernel_spmd  # noqa: E402

B, S, D, H, HD = 4, 2048, 768, 12, 64
HH = H // 2             # 6 heads per core
DH = HH * HD            # 384 ctx dims per core
NPAIR = HH // 2         # 3 head pairs (2 heads packed per 128 partitions)
KC = S // 128           # 16 key chunks
DC = D // 128           # 6 contraction chunks for the projections
NJ = S // 256           # 8 query blocks of 256
NG = 4                  # 4 groups of 512 keys/queries for the projections
N_CORES = 8

F32 = mybir.dt.float32
BF16 = mybir.dt.bfloat16
EXP = mybir.ActivationFunctionType.Exp

LAST_RESULT = None  # BassKernelResults of the most recent run (for test.py)

_CACHED_NC = None


def build_nc():
    nc = bacc.Bacc("TRN2", target_bir_lowering=False)

    xT = nc.dram_tensor("xT", [D, S], BF16, kind="ExternalInput")
    wqT = nc.dram_tensor("wqT", [D, DH], BF16, kind="ExternalInput")
    wkT = nc.dram_tensor("wkT", [D, DH], BF16, kind="ExternalInput")
    wvT = nc.dram_tensor("wvT", [D, DH], BF16, kind="ExternalInput")
    woT = nc.dram_tensor("woT", [DH, D], BF16, kind="ExternalInput")
    tri_d = nc.dram_tensor("tri", [128, 128], BF16, kind="ExternalInput")
    out_d = nc.dram_tensor("out", [S, D], F32, kind="ExternalOutput")

    with tile.TileContext(nc) as tc, ExitStack() as ctx:
        pers = ctx.enter_context(tc.tile_pool(name="pers", bufs=1))
        kT3 = pers.tile([128, NPAIR, S], BF16)          # kT, pair-stacked
        qT3 = pers.tile([128, NPAIR, S], BF16)
        v3 = pers.tile([128, KC, HH, 128], BF16)        # v (+ones col, pad to 128 for FWL) per chunk
        ctx3 = pers.tile([128, NPAIR, S], BF16)         # normalized ctxT
        tri = pers.tile([128, 128], BF16)               # causal k<=u mask
        ones_bf = pers.tile([128, 128], BF16)           # bcast matmul lhsT
        wq_sb = pers.tile([128, DC, DH], BF16)
        wk_sb = pers.tile([128, DC, DH], BF16)
        wv_sb = pers.tile([128, DC, DH], BF16)
        wo_sb = pers.tile([128, NPAIR, D], BF16)

        work = ctx.enter_context(tc.tile_pool(name="work", bufs=1))
        spool = ctx.enter_context(tc.tile_pool(name="spool", bufs=1, space="PSUM"))

        nc.vector.memset(v3[:, :, :, HD], 1.0)          # ones cols, stride 65
        nc.vector.memset(ones_bf, 1.0)
        # DMA order = first-use order: the K projection of group 0 starts
        # after wk chunk 0 + x chunk 0 land, while the rest still streams.
        x_sb0 = work.tile([128, DC, 512], BF16, tag="x", bufs=2, name="x_sb0")
        # split the critical startup loads across two issue queues: wk on
        # the scalar queue, x0 on sync, so descriptor issue (~0.65us each)
        # doesn't serialize the first projection's inputs
        for k in range(DC):
            nc.scalar.dma_start(out=wk_sb[:, k, :], in_=wkT[128 * k:128 * (k + 1), :])
            nc.sync.dma_start(out=x_sb0[:, k, :], in_=xT[128 * k:128 * (k + 1), 0:512])
        for k in range(DC):
            nc.scalar.dma_start(out=wv_sb[:, k, :], in_=wvT[128 * k:128 * (k + 1), :])
        for k in range(DC):
            nc.sync.dma_start(out=wq_sb[:, k, :], in_=wqT[128 * k:128 * (k + 1), :])
        nc.scalar.dma_start(out=tri, in_=tri_d[:])
        for r in range(NPAIR):
            nc.scalar.dma_start(out=wo_sb[:, r, :], in_=woT[128 * r:128 * (r + 1), :])

        pending_norm = []
        pending_ctx = []

        def normalize(r, j, cab):
            """Drain one head-pair/query-block: cast the fused row-sums to
            bf16, broadcast them across partitions with a rank-1 matmul,
            reciprocal the full broadcast tile (approx is exact enough),
            scale, and remap head B to partitions 64-127 via SBUF DMA."""
            jsl = slice(256 * j, 256 * (j + 1))
            rr = work.tile([65, 512], BF16, tag="rr", bufs=3, name="rr")
            nc.vector.tensor_copy(rr[64:65, :], cab[64:65, :])
            # pb borrows an sp-ring slot: those are always free at flush
            # sites, so projection jobs on the 'p' ring never wait on the
            # normalize drain
            pbt = spool.tile([128, 1024], F32, tag="s", bufs=2, name="pbt")
            pb = pbt[:, 0:512]
            nc.tensor.matmul(pb, lhsT=ones_bf[64:65, :], rhs=rr[64:65, :],
                             start=True, stop=True)
            pbr = work.tile([128, 512], F32, tag="pbr", bufs=3, name="pbr")
            nc.vector.reciprocal_approx_fast(pbr, pb)
            nc.vector.tensor_mul(ctx3[0:64, r, jsl], cab[0:64, 0:256],
                                 pbr[0:64, 0:256])
            tB = work.tile([64, 256], BF16, tag="tB", bufs=3, name="tB")
            nc.vector.tensor_mul(tB, cab[0:64, 256:512], pbr[0:64, 256:512])
            nc.sync.dma_start(out=ctx3[64:128, r, jsl], in_=tB)

        def flush_norm():
            while pending_norm:
                r, j, cab = pending_norm.pop(0)
                normalize(r, j, cab)

        def flush_site():
            # drain all but the most recent pending normalize: the newest
            # one's row-sum cast may still be in the vector queue; older
            # ones have had at least a full stream/job of slack
            while len(pending_norm) > 1:
                r, j, cab = pending_norm.pop(0)
                normalize(r, j, cab)

        def flush_ctx():
            # deferred diagonal-pair ctx matmuls: emitted under the matmul
            # cover of the following job/stream so the exp+mask chain of
            # the stream's last pair never stalls the PE queue
            while pending_ctx:
                pending_ctx.pop(0)()

        def attn_stream(j, r):
            flush_site()   # fallback for consecutive streams (cab pressure)
            jsl = slice(256 * j, 256 * (j + 1))
            npairs = j + 1
            if True:
                cab = spool.tile([128, 512], F32, tag="cab", bufs=2, name="cab")
                e_tiles = {}

                def score_mm(sp, p, si, head):
                    a = 2 * p + si
                    asl = slice(128 * a, 128 * (a + 1))
                    zs = 128 if (p == j and si == 1) else 0
                    qsl = slice(256 * j + zs, 256 * (j + 1))
                    # bank layout: [0:512) head-A scores of sites 2p,2p+1
                    # (bank 0); [512:1024) head-B (bank 1).  start=True
                    # clears the whole bank, so only the first matmul per
                    # bank sets it; the second lands as a fresh-element
                    # overwrite with start=False.
                    nc.tensor.matmul(
                        sp[:, 512 * head + 256 * si + zs:
                           512 * head + 256 * (si + 1)],
                        lhsT=kT3[64 * head:64 * head + 64, r, asl],
                        rhs=qT3[64 * head:64 * head + 64, r, qsl],
                        start=(si == 0), stop=True,
                        tile_position=(64 * head, 0), skip_group_check=True)

                def ctx_mm(p, si, head):
                    # lhsT is 128 columns wide (64 hd + ones + 63 pad) so
                    # the compiler's Fast Weight Load triggers (NumWeights
                    # ==128); pad columns accumulate garbage into PSUM
                    # partitions 65-127, which nothing reads
                    e = e_tiles[p]
                    a = 2 * p + si
                    zc = 128 if (p == j and si == 1) else 0
                    nc.tensor.matmul(
                        cab[:, 256 * head + zc:256 * (head + 1)],
                        lhsT=v3[:, a, 2 * r + head, :],
                        rhs=e[:, 512 * head + 256 * si + zc:
                              512 * head + 256 * (si + 1)],
                        start=(a == 0 and head == 0), stop=(a == 2 * j + 1),
                        skip_group_check=True)

                def finish_pair(p):
                    sp = sp_tiles[p]
                    e = work.tile([128, 1024], BF16, tag="e", bufs=6, name="e")
                    nc.scalar.activation(e, sp, EXP, scale=0.125)
                    e_tiles[p] = e
                    if p == j:
                        # partial strips of the two diagonal sites; one
                        # k<=u triangle serves all four, split across the
                        # pool and vector engines so the two chains run in
                        # parallel (~0.9us instead of ~1.7us after exp)
                        for eng, off in ((nc.gpsimd, 0), (nc.vector, 512),
                                         (nc.gpsimd, 384), (nc.vector, 896)):
                            eng.tensor_mul(
                                e[:, off:off + 128], e[:, off:off + 128], tri)

                sp_tiles = {}

                def new_sp(p):
                    sp_tiles[p] = spool.tile([128, 1024], F32, tag="s",
                                             bufs=2, name="sp")

                def emit_scores(p):
                    new_sp(p)
                    for head in range(2):
                        for si in range(2):
                            score_mm(sp_tiles[p], p, si, head)
                    finish_pair(p)

                def emit_ctx(p):
                    for si in range(2):
                        for head in range(2):
                            ctx_mm(p, si, head)

                # software pipeline, lookahead 2: ctx of pair p-2 issues
                # after the scores of pair p, so the exp+mask chain of a
                # pair has two full pairs of tensor work to hide behind.
                LOOK = 2
                for p in range(npairs):
                    emit_scores(p)
                    if p == min(1, npairs - 1):
                        flush_ctx()
                    if p >= LOOK:
                        emit_ctx(p - LOOK)
                for p in range(max(0, npairs - LOOK), npairs - 1):
                    emit_ctx(p)
                pending_ctx.append(lambda: emit_ctx(npairs - 1))
                pending_norm.append((r, j, cab))

        def out_block(j, last=False):
            first = True
            for i in (2 * j, 2 * j + 1):
                isl = slice(128 * i, 128 * (i + 1))
                for lo in (0, DH):
                    po = spool.tile([128, 512], F32, tag="p", bufs=2, name="po")
                    for r in range(NPAIR):
                        nc.tensor.matmul(
                            po[:, 0:DH], lhsT=ctx3[:, r, isl],
                            rhs=wo_sb[:, r, lo:lo + DH],
                            start=(r == 0), stop=(r == NPAIR - 1))
                    if first:
                        flush_ctx()
                        if last:
                            # final block: drain every pending normalize
                            # under this block's matmul cover so the last
                            # out-projection never waits the full chain
                            flush_norm()
                        first = False
                    osb = work.tile([128, DH], F32, tag="osb", bufs=4, name="osb")
                    if last:
                        # scalar engine is idle at the tail; keep the final
                        # drain off the busier vector queue
                        nc.scalar.copy(osb, po[:, 0:DH])
                    else:
                        nc.vector.tensor_copy(osb, po[:, 0:DH])
                    nc.sync.dma_start(out=out_d[isl, lo:lo + DH], in_=osb)
            flush_site()

        def kq_job(w_sb, dstT, r, x_sb, gsl):
            ps = spool.tile([128, 512], F32, tag="p", bufs=2, name="pskq")
            for k in range(DC):
                nc.tensor.matmul(
                    ps, lhsT=w_sb[:, k, 128 * r:128 * (r + 1)],
                    rhs=x_sb[:, k, :], start=(k == 0), stop=(k == DC - 1))
            flush_ctx()
            nc.vector.tensor_copy(dstT[:, r, gsl], ps)
            flush_site()

        def kq_pair(w0, d0, w1, d1, r, x_sb, gsl):
            # K and Q projections for one head-pair with interleaved
            # accumulation groups (alternating PSUM banks)
            ps0 = spool.tile([128, 512], F32, tag="p", bufs=2, name="pq0")
            ps1 = spool.tile([128, 512], F32, tag="p", bufs=2, name="pq1")
            for k in range(DC):
                nc.tensor.matmul(
                    ps0, lhsT=w0[:, k, 128 * r:128 * (r + 1)],
                    rhs=x_sb[:, k, :], start=(k == 0), stop=(k == DC - 1),
                    skip_group_check=True)
                nc.tensor.matmul(
                    ps1, lhsT=w1[:, k, 128 * r:128 * (r + 1)],
                    rhs=x_sb[:, k, :], start=(k == 0), stop=(k == DC - 1),
                    skip_group_check=True)
            flush_ctx()
            nc.vector.tensor_copy(d0[:, r, gsl], ps0)
            nc.vector.tensor_copy(d1[:, r, gsl], ps1)
            flush_site()

        def v_job(a, aa, x_sb):
            ps = spool.tile([128, 512], F32, tag="p", bufs=2, name="psv")
            for k in range(DC):
                nc.tensor.matmul(
                    ps[:, 0:DH], lhsT=x_sb[:, k, 128 * aa:128 * (aa + 1)],
                    rhs=wv_sb[:, k, :], start=(k == 0), stop=(k == DC - 1))
            flush_ctx()
            nc.vector.tensor_copy(
                v3[:, a, :, 0:HD],
                ps[:, 0:DH].rearrange("p (h e) -> p h e", e=HD))
            flush_site()

        # projection jobs are threaded between attention streams: the
        # scalar engine's exp backlog drains while the PE runs projection
        # matmuls, and every job boundary is a normalize flush site
        x_sb = x_sb0
        for g in range(NG):
            gsl = slice(512 * g, 512 * (g + 1))
            if g == 0:
                kq_job(wk_sb, kT3, 0, x_sb, gsl)
            if g + 1 < NG:
                x_nxt = work.tile([128, DC, 512], BF16, tag="x", bufs=2,
                                  name="x_nxt")
                for k in range(DC):
                    nc.sync.dma_start(
                        out=x_nxt[:, k, :],
                        in_=xT[128 * k:128 * (k + 1),
                               512 * (g + 1):512 * (g + 2)])
            v_job(4 * g, 0, x_sb)
            v_job(4 * g + 1, 1, x_sb)
            if g == 0:
                kq_job(wq_sb, qT3, 0, x_sb, gsl)
            attn_stream(2 * g, 0)
            kq_pair(wk_sb, kT3, wq_sb, qT3, 1, x_sb, gsl)
            attn_stream(2 * g, 1)
            kq_pair(wk_sb, kT3, wq_sb, qT3, 2, x_sb, gsl)
            attn_stream(2 * g, 2)
            v_job(4 * g + 2, 2, x_sb)
            v_job(4 * g + 3, 3, x_sb)
            attn_stream(2 * g + 1, 0)
            if g > 0:
                out_block(2 * g - 1)
            if g + 1 < NG:
                # next group's first Q projection, also hoisted
                kq_job(wq_sb, qT3, 0, x_nxt,
                       slice(512 * (g + 1), 512 * (g + 2)))
            attn_stream(2 * g + 1, 1)
            if g + 1 < NG:
                # next group's first K projection, hoisted into the
                # attention-dense stretch
                kq_job(wk_sb, kT3, 0, x_nxt,
                       slice(512 * (g + 1), 512 * (g + 2)))
            attn_stream(2 * g + 1, 2)
            out_block(2 * g, last=(g == NG - 1))
            if g + 1 < NG:
                x_sb = x_nxt

        out_block(NJ - 1)

    nc.compile()
    return nc


def get_nc():
    global _CACHED_NC
    if _CACHED_NC is None:
        _CACHED_NC = build_nc()
    return _CACHED_NC


def make_core_inputs(x, wq, wk, wv, wo):
    """Host-side shard prep: slices/transposes/dtype rounding only."""
    import ml_dtypes
    bf16 = ml_dtypes.bfloat16

    tri = (np.arange(128)[:, None] <= np.arange(128)[None, :]).astype(bf16)

    wslices = []
    for hh in range(2):
        hsl = slice(DH * hh, DH * (hh + 1))
        wslices.append({
            "wqT": np.ascontiguousarray(wq[hsl, :].T.astype(bf16)),
            "wkT": np.ascontiguousarray(wk[hsl, :].T.astype(bf16)),
            "wvT": np.ascontiguousarray(wv[hsl, :].T.astype(bf16)),
            "woT": np.ascontiguousarray(wo[:, hsl].T.astype(bf16)),
        })

    in_maps = []
    for c in range(N_CORES):
        b, hh = c // 2, c % 2
        xT_b = np.ascontiguousarray(x[b].T.astype(bf16))
        m = {"xT": xT_b, "tri": tri}
        m.update(wslices[hh])
        in_maps.append(m)
    return in_maps


def kernel(x, wq, wk, wv, wo, bo):
    global LAST_RESULT
    x = np.asarray(x, np.float32)
    bo = np.asarray(bo, np.float32)
    in_maps = make_core_inputs(
        x, np.asarray(wq, np.float32), np.asarray(wk, np.float32),
        np.asarray(wv, np.float32), np.asarray(wo, np.float32))

    nc = get_nc()
    trace = bool(int(os.environ.get("KERNEL_TRACE", "0")))
    kwargs = {}
    if trace:
        kwargs.update(trace=True, trace_cores=[0, 1],
                      tmpdir=os.environ.get("KERNEL_TRACE_DIR") or None)
    res = run_bass_kernel_spmd(nc, in_maps, list(range(N_CORES)), **kwargs)
    LAST_RESULT = res

    out = np.empty((B, S, D), np.float32)
    for b in range(B):
        out[b] = res.results[2 * b]["out"] + res.results[2 * b + 1]["out"] + bo
    return out


# revision 56
# speedup vs baseline: 1.0068x; 1.0068x over previous
"""Trainium2 Bass kernel for 12-head causal MHA (B=4, S=2048, D=768).

Sharding: 8 cores, core c -> (batch c//2, head-half c%2).  Each core
computes 6 heads over ALL 2048 queries of its batch and emits the
PARTIAL out-projection (its 384 ctx dims x woT slice); the host sums
the two half-partials per batch and adds the bias.  This removes the
K/V-projection duplication of batch x query-parity sharding and makes
queries contiguous (simple causal masks).

Layout is fully transposed so every matmul contracts along partitions:
  qT/kT: [head_dim, seq]  scoresT: [sk, sq]  ctxT: [hd+1, sq]
The softmax row-sum is fused into the ctx matmul via a ones column
appended to V (M=65).  Softmax skips max-subtraction (scores/8 are
bounded by ~2 for this distribution, exp is safe).

Schedule: projection jobs (512-key groups), attention streams (one
head-pair x 256-query block) and the out-projection are threaded into
one instruction stream so the PE never idles long enough to drop out of
its max p-state and the scalar engine's exp backlog drains during
projection bursts.  The attention inner loop is software-pipelined with
lookahead 2 (ctx of pair p-2 issues after the scores of pair p, hiding
the exp+mask latency), scores matmuls pack both heads into PE quadrant
pairs (tile_position row split), and causal masking multiplies a single
k<=u triangle on the (otherwise idle) gpsimd engine.  Softmax
normalization (bf16 row-sum cast -> rank-1 broadcast matmul ->
full-tile approx reciprocal -> scale) is deferred by at least one full
stream/job so its only tensor instruction never stalls the PE queue.
"""

import os
import sys
from contextlib import ExitStack

import numpy as np

os.environ.setdefault("MYCRO_LOCAL_CACHE", "1")

for _p in ("/root/.axon_site/_ro/trn_rl_repo", "/opt/trn_rl_repo"):
    # later inserts win: prefer /opt (writable sibling modules, e.g.
    # antenv.axon_hooks) over the read-only mirror
    if os.path.isdir(_p) and _p not in sys.path:
        sys.path.insert(0, _p)

import concourse.bass as bass  # noqa: E402
import concourse.tile as tile  # noqa: E402
from concourse import bacc, mybir  # noqa: E402
from concourse.bass_utils import run_bass_kernel_spmd  # noqa: E402

B, S, D, H, HD = 4, 2048, 768, 12, 64
HH = H // 2             # 6 heads per core
DH = HH * HD            # 384 ctx dims per core
NPAIR = HH // 2         # 3 head pairs (2 heads packed per 128 partitions)
KC = S // 128           # 16 key chunks
DC = D // 128           # 6 contraction chunks for the projections
NJ = S // 256           # 8 query blocks of 256
NG = 4                  # 4 groups of 512 keys/queries for the projections
N_CORES = 8

F32 = mybir.dt.float32
BF16 = mybir.dt.bfloat16
EXP = mybir.ActivationFunctionType.Exp

LAST_RESULT = None  # BassKernelResults of the most recent run (for test.py)

_CACHED_NC = None


def build_nc():
    nc = bacc.Bacc("TRN2", target_bir_lowering=False)

    xT = nc.dram_tensor("xT", [D, S], BF16, kind="ExternalInput")
    wqT = nc.dram_tensor("wqT", [D, DH], BF16, kind="ExternalInput")
    wkT = nc.dram_tensor("wkT", [D, DH], BF16, kind="ExternalInput")
    wvT = nc.dram_tensor("wvT", [D, DH], BF16, kind="ExternalInput")
    woT = nc.dram_tensor("woT", [DH, D], BF16, kind="ExternalInput")
    tri_d = nc.dram_tensor("tri", [128, 128], BF16, kind="ExternalInput")
    out_d = nc.dram_tensor("out", [S, D], F32, kind="ExternalOutput")

    with tile.TileContext(nc) as tc, ExitStack() as ctx:
        pers = ctx.enter_context(tc.tile_pool(name="pers", bufs=1))
        kT3 = pers.tile([128, NPAIR, S], BF16)          # kT, pair-stacked
        qT3 = pers.tile([128, NPAIR, S], BF16)
        v3 = pers.tile([128, KC, HH, 128], BF16)        # v (+ones col, pad to 128 for FWL) per chunk
        ctx3 = pers.tile([128, NPAIR, S], BF16)         # normalized ctxT
        tri = pers.tile([128, 128], BF16)               # causal k<=u mask
        ones_bf = pers.tile([128, 128], BF16)           # bcast matmul lhsT
        wq_sb = pers.tile([128, DC, DH], BF16)
        wk_sb = pers.tile([128, DC, DH], BF16)
        wv_sb = pers.tile([128, DC, DH], BF16)
        wo_sb = pers.tile([128, NPAIR, D], BF16)

        work = ctx.enter_context(tc.tile_pool(name="work", bufs=1))
        spool = ctx.enter_context(tc.tile_pool(name="spool", bufs=1, space="PSUM"))

        nc.vector.memset(v3[:, :, :, HD], 1.0)          # ones cols, stride 65
        nc.vector.memset(ones_bf, 1.0)
        # DMA order = first-use order: the K projection of group 0 starts
        # after wk chunk 0 + x chunk 0 land, while the rest still streams.
        x_sb0 = work.tile([128, DC, 512], BF16, tag="x", bufs=2, name="x_sb0")
        # split the critical startup loads across two issue queues: wk on
        # the scalar queue, x0 on sync, so descriptor issue (~0.65us each)
        # doesn't serialize the first projection's inputs
        for k in range(DC):
            nc.scalar.dma_start(out=wk_sb[:, k, :], in_=wkT[128 * k:128 * (k + 1), :])
            nc.sync.dma_start(out=x_sb0[:, k, :], in_=xT[128 * k:128 * (k + 1), 0:512])
        for k in range(DC):
            nc.scalar.dma_start(out=wv_sb[:, k, :], in_=wvT[128 * k:128 * (k + 1), :])
        for k in range(DC):
            nc.sync.dma_start(out=wq_sb[:, k, :], in_=wqT[128 * k:128 * (k + 1), :])
        nc.scalar.dma_start(out=tri, in_=tri_d[:])
        for r in range(NPAIR):
            nc.scalar.dma_start(out=wo_sb[:, r, :], in_=woT[128 * r:128 * (r + 1), :])

        pending_norm = []
        pending_ctx = []

        def normalize(r, j, cab):
            """Drain one head-pair/query-block: cast the fused row-sums to
            bf16, broadcast them across partitions with a rank-1 matmul,
            reciprocal the full broadcast tile (approx is exact enough),
            scale, and remap head B to partitions 64-127 via SBUF DMA."""
            jsl = slice(256 * j, 256 * (j + 1))
            rr = work.tile([65, 512], BF16, tag="rr", bufs=3, name="rr")
            nc.vector.tensor_copy(rr[64:65, :], cab[64:65, :])
            # pb borrows an sp-ring slot: those are always free at flush
            # sites, so projection jobs on the 'p' ring never wait on the
            # normalize drain
            pbt = spool.tile([128, 1024], F32, tag="s", bufs=2, name="pbt")
            pb = pbt[:, 0:512]
            nc.tensor.matmul(pb, lhsT=ones_bf[64:65, :], rhs=rr[64:65, :],
                             start=True, stop=True)
            pbr = work.tile([128, 512], F32, tag="pbr", bufs=3, name="pbr")
            nc.vector.reciprocal_approx_fast(pbr, pb)
            nc.vector.tensor_mul(ctx3[0:64, r, jsl], cab[0:64, 0:256],
                                 pbr[0:64, 0:256])
            tB = work.tile([64, 256], BF16, tag="tB", bufs=3, name="tB")
            nc.vector.tensor_mul(tB, cab[0:64, 256:512], pbr[0:64, 256:512])
            nc.sync.dma_start(out=ctx3[64:128, r, jsl], in_=tB)

        def flush_norm():
            while pending_norm:
                r, j, cab = pending_norm.pop(0)
                normalize(r, j, cab)

        def flush_site():
            # drain all but the most recent pending normalize: the newest
            # one's row-sum cast may still be in the vector queue; older
            # ones have had at least a full stream/job of slack
            while len(pending_norm) > 1:
                r, j, cab = pending_norm.pop(0)
                normalize(r, j, cab)

        def flush_ctx():
            # deferred diagonal-pair ctx matmuls: emitted under the matmul
            # cover of the following job/stream so the exp+mask chain of
            # the stream's last pair never stalls the PE queue
            while pending_ctx:
                pending_ctx.pop(0)()

        def attn_stream(j, r):
            flush_site()   # fallback for consecutive streams (cab pressure)
            jsl = slice(256 * j, 256 * (j + 1))
            npairs = j + 1
            if True:
                cab = spool.tile([128, 512], F32, tag="cab", bufs=2, name="cab")
                e_tiles = {}

                def score_mm(sp, p, si, head):
                    a = 2 * p + si
                    asl = slice(128 * a, 128 * (a + 1))
                    zs = 128 if (p == j and si == 1) else 0
                    qsl = slice(256 * j + zs, 256 * (j + 1))
                    # bank layout: [0:512) head-A scores of sites 2p,2p+1
                    # (bank 0); [512:1024) head-B (bank 1).  start=True
                    # clears the whole bank, so only the first matmul per
                    # bank sets it; the second lands as a fresh-element
                    # overwrite with start=False.
                    nc.tensor.matmul(
                        sp[:, 512 * head + 256 * si + zs:
                           512 * head + 256 * (si + 1)],
                        lhsT=kT3[64 * head:64 * head + 64, r, asl],
                        rhs=qT3[64 * head:64 * head + 64, r, qsl],
                        start=(si == 0), stop=True,
                        tile_position=(64 * head, 0), skip_group_check=True)

                def ctx_mm(p, si, head):
                    # lhsT is 128 columns wide (64 hd + ones + 63 pad) so
                    # the compiler's Fast Weight Load triggers (NumWeights
                    # ==128); pad columns accumulate garbage into PSUM
                    # partitions 65-127, which nothing reads
                    e = e_tiles[p]
                    a = 2 * p + si
                    zc = 128 if (p == j and si == 1) else 0
                    nc.tensor.matmul(
                        cab[:, 256 * head + zc:256 * (head + 1)],
                        lhsT=v3[:, a, 2 * r + head, :],
                        rhs=e[:, 512 * head + 256 * si + zc:
                              512 * head + 256 * (si + 1)],
                        start=(a == 0 and head == 0), stop=(a == 2 * j + 1),
                        skip_group_check=True)

                def finish_pair(p):
                    sp = sp_tiles[p]
                    e = work.tile([128, 1024], BF16, tag="e", bufs=6, name="e")
                    nc.scalar.activation(e, sp, EXP, scale=0.125)
                    e_tiles[p] = e
                    if p == j:
                        # partial strips of the two diagonal sites; one
                        # k<=u triangle serves all four, split across the
                        # pool and vector engines so the two chains run in
                        # parallel (~0.9us instead of ~1.7us after exp)
                        for eng, off in ((nc.gpsimd, 0), (nc.vector, 512),
                                         (nc.gpsimd, 384), (nc.vector, 896)):
                            eng.tensor_mul(
                                e[:, off:off + 128], e[:, off:off + 128], tri)

                sp_tiles = {}

                def new_sp(p):
                    sp_tiles[p] = spool.tile([128, 1024], F32, tag="s",
                                             bufs=2, name="sp")

                def emit_scores(p):
                    new_sp(p)
                    for head in range(2):
                        for si in range(2):
                            score_mm(sp_tiles[p], p, si, head)
                    finish_pair(p)

                def emit_ctx(p):
                    for si in range(2):
                        for head in range(2):
                            ctx_mm(p, si, head)

                # software pipeline, lookahead 2: ctx of pair p-2 issues
                # after the scores of pair p, so the exp+mask chain of a
                # pair has two full pairs of tensor work to hide behind.
                LOOK = 2
                for p in range(npairs):
                    emit_scores(p)
                    if p == min(1, npairs - 1):
                        flush_ctx()
                    if p >= LOOK:
                        emit_ctx(p - LOOK)
                for p in range(max(0, npairs - LOOK), npairs - 1):
                    emit_ctx(p)
                pending_ctx.append(lambda: emit_ctx(npairs - 1))
                pending_norm.append((r, j, cab))

        def out_block(j, last=False):
            first = True
            for i in (2 * j, 2 * j + 1):
                isl = slice(128 * i, 128 * (i + 1))
                for lo in (0, DH):
                    po = spool.tile([128, 512], F32, tag="p", bufs=2, name="po")
                    for r in range(NPAIR):
                        nc.tensor.matmul(
                            po[:, 0:DH], lhsT=ctx3[:, r, isl],
                            rhs=wo_sb[:, r, lo:lo + DH],
                            start=(r == 0), stop=(r == NPAIR - 1))
                    if first:
                        flush_ctx()
                        if last:
                            # final block: drain every pending normalize
                            # under this block's matmul cover so the last
                            # out-projection never waits the full chain
                            flush_norm()
                        first = False
                    osb = work.tile([128, DH], F32, tag="osb", bufs=4, name="osb")
                    if last:
                        # scalar engine is idle at the tail; keep the final
                        # drain off the busier vector queue
                        nc.scalar.copy(osb, po[:, 0:DH])
                    else:
                        nc.vector.tensor_copy(osb, po[:, 0:DH])
                    nc.sync.dma_start(out=out_d[isl, lo:lo + DH], in_=osb)
            flush_site()

        def kq_job(w_sb, dstT, r, x_sb, gsl):
            ps = spool.tile([128, 512], F32, tag="p", bufs=2, name="pskq")
            for k in range(DC):
                nc.tensor.matmul(
                    ps, lhsT=w_sb[:, k, 128 * r:128 * (r + 1)],
                    rhs=x_sb[:, k, :], start=(k == 0), stop=(k == DC - 1))
            flush_ctx()
            nc.vector.tensor_copy(dstT[:, r, gsl], ps)
            flush_site()

        def v_job(a, aa, x_sb):
            ps = spool.tile([128, 512], F32, tag="p", bufs=2, name="psv")
            for k in range(DC):
                nc.tensor.matmul(
                    ps[:, 0:DH], lhsT=x_sb[:, k, 128 * aa:128 * (aa + 1)],
                    rhs=wv_sb[:, k, :], start=(k == 0), stop=(k == DC - 1))
            flush_ctx()
            nc.vector.tensor_copy(
                v3[:, a, :, 0:HD],
                ps[:, 0:DH].rearrange("p (h e) -> p h e", e=HD))
            flush_site()

        # projection jobs are threaded between attention streams: the
        # scalar engine's exp backlog drains while the PE runs projection
        # matmuls, and every job boundary is a normalize flush site
        x_sb = x_sb0
        for g in range(NG):
            gsl = slice(512 * g, 512 * (g + 1))
            if g == 0:
                kq_job(wk_sb, kT3, 0, x_sb, gsl)
            if g + 1 < NG:
                x_nxt = work.tile([128, DC, 512], BF16, tag="x", bufs=2,
                                  name="x_nxt")
                for k in range(DC):
                    nc.sync.dma_start(
                        out=x_nxt[:, k, :],
                        in_=xT[128 * k:128 * (k + 1),
                               512 * (g + 1):512 * (g + 2)])
            v_job(4 * g, 0, x_sb)
            v_job(4 * g + 1, 1, x_sb)
            if g == 0:
                kq_job(wq_sb, qT3, 0, x_sb, gsl)
            attn_stream(2 * g, 0)
            kq_job(wk_sb, kT3, 1, x_sb, gsl)
            kq_job(wq_sb, qT3, 1, x_sb, gsl)
            attn_stream(2 * g, 1)
            kq_job(wk_sb, kT3, 2, x_sb, gsl)
            kq_job(wq_sb, qT3, 2, x_sb, gsl)
            attn_stream(2 * g, 2)
            v_job(4 * g + 2, 2, x_sb)
            v_job(4 * g + 3, 3, x_sb)
            attn_stream(2 * g + 1, 0)
            if g > 0:
                out_block(2 * g - 1)
            if g + 1 < NG:
                # next group's first Q projection, also hoisted
                kq_job(wq_sb, qT3, 0, x_nxt,
                       slice(512 * (g + 1), 512 * (g + 2)))
            attn_stream(2 * g + 1, 1)
            if g + 1 < NG:
                # next group's first K projection, hoisted into the
                # attention-dense stretch
                kq_job(wk_sb, kT3, 0, x_nxt,
                       slice(512 * (g + 1), 512 * (g + 2)))
            attn_stream(2 * g + 1, 2)
            out_block(2 * g, last=(g == NG - 1))
            if g + 1 < NG:
                x_sb = x_nxt

        out_block(NJ - 1)

    nc.compile()
    return nc


def get_nc():
    global _CACHED_NC
    if _CACHED_NC is None:
        _CACHED_NC = build_nc()
    return _CACHED_NC


def make_core_inputs(x, wq, wk, wv, wo):
    """Host-side shard prep: slices/transposes/dtype rounding only."""
    import ml_dtypes
    bf16 = ml_dtypes.bfloat16

    tri = (np.arange(128)[:, None] <= np.arange(128)[None, :]).astype(bf16)

    wslices = []
    for hh in range(2):
        hsl = slice(DH * hh, DH * (hh + 1))
        wslices.append({
            "wqT": np.ascontiguousarray(wq[hsl, :].T.astype(bf16)),
            "wkT": np.ascontiguousarray(wk[hsl, :].T.astype(bf16)),
            "wvT": np.ascontiguousarray(wv[hsl, :].T.astype(bf16)),
            "woT": np.ascontiguousarray(wo[:, hsl].T.astype(bf16)),
        })

    in_maps = []
    for c in range(N_CORES):
        b, hh = c // 2, c % 2
        xT_b = np.ascontiguousarray(x[b].T.astype(bf16))
        m = {"xT": xT_b, "tri": tri}
        m.update(wslices[hh])
        in_maps.append(m)
    return in_maps


def kernel(x, wq, wk, wv, wo, bo):
    global LAST_RESULT
    x = np.asarray(x, np.float32)
    bo = np.asarray(bo, np.float32)
    in_maps = make_core_inputs(
        x, np.asarray(wq, np.float32), np.asarray(wk, np.float32),
        np.asarray(wv, np.float32), np.asarray(wo, np.float32))

    nc = get_nc()
    trace = bool(int(os.environ.get("KERNEL_TRACE", "0")))
    kwargs = {}
    if trace:
        kwargs.update(trace=True, trace_cores=[0, 1],
                      tmpdir=os.environ.get("KERNEL_TRACE_DIR") or None)
    res = run_bass_kernel_spmd(nc, in_maps, list(range(N_CORES)), **kwargs)
    LAST_RESULT = res

    out = np.empty((B, S, D), np.float32)
    for b in range(B):
        out[b] = res.results[2 * b]["out"] + res.results[2 * b + 1]["out"] + bo
    return out
